# revision 24
# baseline (speedup 1.0000x reference)
"""GCN-VAE forward as a distributed 8-core Trainium2 Bass kernel.

Sharding: nodes (rows of adj / all [N,*] activations) are split across the
8 cores, 1024 rows each.  adj and x are fed pre-transposed (adjT / xT column
shards) so every matmul contracts over the partition axis without on-the-fly
DRAM transposes.  Heavy matmuls run in bf16 with fp32 PSUM accumulation.

Structure per core:
  warmup AllGather (absorbs inter-core start skew) || adjT resident load
  pass0: xw = x@W1 shard -> AllGather
  pass1: h1T = (xw^T contracted with adjT) -> relu -> PE-transpose -> AllGather
  pass2: sT = (adj@h1)^T shard;  m4T = Wcat^T . sT  (mu/logvar/clz_mu/clz_logvar)
  z -> ub = WlA^T zT -> AllGather(ub)  [issued before the stats AllReduce so the
  reduce hides under the gather];  uz = ub + const_row applied post-gather
  zz: adj_recon row shard = uz_shard @ uz_full^T (bf16)

Device outputs per core (host reassembles / transposes):
  recon [1024, 8192] f32; zt/mut/lvt [128, 1024] f32; grp [128, 2] f32
"""

import sys

for _p in ("/opt/trn_rl_repo",):
    if _p not in sys.path:
        sys.path.insert(0, _p)

import numpy as np
import ml_dtypes

import concourse.bass as bass
import concourse.bacc as bacc
import concourse.tile as tile
import concourse.mybir as mybir
from concourse import bass_utils, masks

F32 = mybir.dt.float32
BF16 = mybir.dt.bfloat16
BF16_NP = ml_dtypes.bfloat16

N = 8192          # nodes
NCORE = 8
S = N // NCORE    # 1024 rows per core
FIN = 512
H1 = 256
H2 = 128
P = 128           # partitions
KCH = N // P      # 64 contraction chunks over nodes
SB = S // P       # 8 row blocks per core
RES = 44          # adjT chunks kept resident in SBUF (rest streamed per pass)

_CACHED_NC = None


def _build():
    nc = bacc.Bacc(
        "TRN2",
        target_bir_lowering=False,
        debug=False,
        enable_asserts=True,
        num_devices=NCORE,
    )

    # ---- per-core external inputs ----
    adjt = nc.dram_tensor("adjt", [N, S], BF16, kind="ExternalInput")
    xt = nc.dram_tensor("xt", [FIN, N], BF16, kind="ExternalInput")
    w1 = nc.dram_tensor("w1", [FIN, H1], BF16, kind="ExternalInput")
    wcat = nc.dram_tensor("wcat", [H1, 4 * H2], BF16, kind="ExternalInput")
    wla = nc.dram_tensor("wla", [H2, H2], BF16, kind="ExternalInput")
    wlb = nc.dram_tensor("wlb", [H2, H2], F32, kind="ExternalInput")
    blc = nc.dram_tensor("blc", [H2, 1], F32, kind="ExternalInput")
    epszt = nc.dram_tensor("epszt", [H2, S], F32, kind="ExternalInput")
    epsgt = nc.dram_tensor("epsgt", [H2, 1], F32, kind="ExternalInput")

    # ---- per-core external outputs ----
    recon = nc.dram_tensor("recon", [S, N], F32, kind="ExternalOutput")
    zt_o = nc.dram_tensor("zt", [H2, S], F32, kind="ExternalOutput")
    mut_o = nc.dram_tensor("mut", [H2, S], F32, kind="ExternalOutput")
    lvt_o = nc.dram_tensor("lvt", [H2, S], F32, kind="ExternalOutput")
    grp_o = nc.dram_tensor("grp", [H2, 2], F32, kind="ExternalOutput")

    RG = [list(range(NCORE))]
    AX = mybir.AxisListType.X
    AF = mybir.ActivationFunctionType

    with tile.TileContext(nc) as tc:
        with (
            tc.tile_pool(name="const", bufs=1) as constp,
            tc.tile_pool(name="persist", bufs=1) as persist,
            tc.tile_pool(name="drain", bufs=3) as drainp,
            tc.tile_pool(name="psacc", bufs=1, space="PSUM") as psacc,
            tc.tile_pool(name="pssm", bufs=4, space="PSUM") as pssm,
            tc.tile_pool(name="dram", bufs=1, space="DRAM") as dramp,
        ):
            # collective buffers
            h1_bnc = dramp.tile([S, H1], BF16, tag="h1_bnc")
            h1_g = dramp.tile([N, H1], BF16, tag="h1_g", addr_space="Shared")
            uz_bnc = dramp.tile([H2, S + 4], BF16, tag="uz_bnc")
            uz_g = dramp.tile([NCORE * H2, S + 4], BF16, tag="uz_g",
                              addr_space="Shared")

            # big feature slot reused xw_full -> h1_full
            def bigfeat(dtype, cols, name):
                return persist.tile([P, cols], dtype, tag="bigfeat", name=name)

            h1t_sb = [
                persist.tile([P, S], BF16, tag=f"h1t{jb}", name=f"h1t_sb{jb}")
                for jb in range(2)
            ]
            sT_sb = persist.tile([P, 2 * S], BF16, tag="sT")

            with (
                tc.tile_pool(name="adjres", bufs=1) as adjp,
                tc.tile_pool(name="adjstream", bufs=12) as adjsp,
            ):
                # ---------- adjT resident load first, on the gpsimd DGE ----------
                adj_res = []
                with nc.named_scope("adj_load"):
                    for c in range(RES):
                        t = adjp.tile([P, S], BF16, tag=f"adj{c}", name=f"adj{c}")
                        nc.gpsimd.dma_start(t[:], adjt[c * P:(c + 1) * P, :])
                        adj_res.append(t)

                def adj_chunk(c, phase):
                    if c < RES:
                        return adj_res[c]
                    t = adjsp.tile([P, S], BF16, tag="adjstream",
                                   name=f"adjs_{phase}_{c}")
                    nc.gpsimd.dma_start(t[:], adjt[c * P:(c + 1) * P, :])
                    return t

                # ---------- constants (small, on the sync DGE) ----------
                ident_bf = constp.tile([P, P], BF16, tag="identb")
                masks.make_identity(nc, ident_bf[:])
                w1_sb = constp.tile([P, 4 * H1], BF16, tag="w1")
                for c in range(4):
                    nc.sync.dma_start(
                        w1_sb[:, c * H1:(c + 1) * H1], w1[c * P:(c + 1) * P, :]
                    )
                wcat_sb = constp.tile([P, 2 * 512], BF16, tag="wcat")
                for c in range(2):
                    nc.scalar.dma_start(
                        wcat_sb[:, c * 512:(c + 1) * 512], wcat[c * P:(c + 1) * P, :]
                    )
                wla_sb = constp.tile([P, H2], BF16, tag="wla")
                nc.scalar.dma_start(wla_sb[:], wla[:, :])
                wlb_sb = constp.tile([P, H2], F32, tag="wlb")
                nc.scalar.dma_start(wlb_sb[:], wlb[:, :])
                blc_sb = constp.tile([P, 1], F32, tag="blc")
                nc.scalar.dma_start(blc_sb[:], blc[:, :])
                epsg_sb = constp.tile([P, 1], F32, tag="epsg")
                nc.scalar.dma_start(epsg_sb[:], epsgt[:, :])
                epszt_sb = constp.tile([P, S], F32, tag="epszt")
                nc.scalar.dma_start(epszt_sb[:], epszt[:, :])

                # ---------- fused pass0+pass1 ----------
                # pass0 chunk c produces xw rows [c*128,(c+1)*128); pass1 chunk
                # c consumes them immediately (software-pipelined by one chunk
                # so the PE never waits on the ACT drain).
                xw_full = bigfeat(BF16, KCH * H1, "xw_full")
                h1t_ps = [
                    psacc.tile([P, S], F32, tag=f"accT{jb}", name=f"h1t_ps{jb}")
                    for jb in range(2)
                ]
                with tc.tile_pool(name="xtpool", bufs=3) as xtp:
                    NG = 8  # column groups of 1024 nodes
                    GW = N // NG
                    xt_tiles = {}

                    def xt_group(g):
                        if g in xt_tiles:
                            return xt_tiles[g]
                        xg = xtp.tile([P, 4 * GW], BF16, tag="xtg",
                                      name=f"xt{g}")
                        for c in range(4):
                            nc.sync.dma_start(
                                xg[:, c * GW:(c + 1) * GW],
                                xt[c * P:(c + 1) * P, g * GW:(g + 1) * GW],
                            )
                        xt_tiles[g] = xg
                        return xg

                    def pass0_chunk(ib):
                        g, off = divmod(ib * P, GW)
                        # prefetch this and the next group
                        xg = xt_group(g)
                        if g + 1 < NG and off == 0:
                            xt_group(g + 1)
                        ps = pssm.tile([P, H1], F32, tag="sm", name=f"ps0_{ib}")
                        for kc in range(4):
                            nc.tensor.matmul(
                                ps[:],
                                lhsT=xg[:, kc * GW + off: kc * GW + off + P],
                                rhs=w1_sb[:, kc * H1:(kc + 1) * H1],
                                start=(kc == 0),
                                stop=(kc == 3),
                            )
                        nc.scalar.copy(xw_full[:, ib * H1:(ib + 1) * H1], ps[:])

                    def pass1_chunk(c):
                        a = adj_chunk(c, 1)
                        for jb in range(2):
                            for hf in range(2):
                                nc.tensor.matmul(
                                    h1t_ps[jb][:, hf * 512:(hf + 1) * 512],
                                    lhsT=xw_full[:, c * H1 + jb * P: c * H1 + (jb + 1) * P],
                                    rhs=a[:, hf * 512:(hf + 1) * 512],
                                    start=(c == 0),
                                    stop=(c == KCH - 1),
                                )

                    with nc.named_scope("p01"):
                        for c in range(KCH + 1):
                            if c < KCH:
                                pass0_chunk(c)
                            if c >= 1:
                                pass1_chunk(c - 1)

                for jb in range(2):
                    nc.scalar.activation(h1t_sb[jb][:], h1t_ps[jb][:], AF.Relu)

                # transpose h1T -> h1 natural [1024, 256], all-gather
                for b in range(SB):
                    h1n = drainp.tile([P, H1], BF16, tag="h1n", name=f"h1n{b}")
                    for jb in range(2):
                        tp = pssm.tile([P, P], BF16, tag="sm", name=f"tp{b}_{jb}")
                        nc.tensor.transpose(
                            tp[:], h1t_sb[jb][:, b * P:(b + 1) * P], ident_bf[:]
                        )
                        nc.scalar.copy(h1n[:, jb * P:(jb + 1) * P], tp[:])
                    nc.sync.dma_start(h1_bnc[b * P:(b + 1) * P, :], h1n[:])

                nc.gpsimd.collective_compute(
                    "AllGather", mybir.AluOpType.bypass, replica_groups=RG,
                    ins=[h1_bnc[:].opt()], outs=[h1_g[:].opt()],
                )
                h1_full = bigfeat(BF16, KCH * H1, "h1_full")
                for b in range(8):
                    srcap = h1_g[b * 8 * P:(b + 1) * 8 * P, :].rearrange(
                        "(c p) j -> p c j", p=P
                    )
                    dstap = h1_full[:, b * 8 * H1:(b + 1) * 8 * H1].rearrange(
                        "p (c j) -> p c j", c=8
                    )
                    eng = nc.scalar if b % 2 == 0 else nc.sync
                    eng.dma_start(dstap, srcap)

                # ---------- pass 2: sT[j,i] = sum_k h1[k,j] adjT[k,i] ----------
                st_ps = [
                    psacc.tile([P, S], F32, tag=f"accT{jb}", name=f"st_ps{jb}")
                    for jb in range(2)
                ]
                with nc.named_scope("pass2"):
                    for c in range(KCH):
                        a = adj_chunk(c, 2)
                        for jb in range(2):
                            for hf in range(2):
                                nc.tensor.matmul(
                                    st_ps[jb][:, hf * 512:(hf + 1) * 512],
                                    lhsT=h1_full[:, c * H1 + jb * P: c * H1 + (jb + 1) * P],
                                    rhs=a[:, hf * 512:(hf + 1) * 512],
                                    start=(c == 0),
                                    stop=(c == KCH - 1),
                                )
                for jb in range(2):
                    nc.scalar.copy(sT_sb[:, jb * S:(jb + 1) * S], st_ps[jb][:])

            # ---------- tail ----------
            with (
                tc.tile_pool(name="tail", bufs=1) as tailp,
                tc.tile_pool(name="rcpool", bufs=3) as rcpool,
            ):
                mut_sb = tailp.tile([P, S], F32, tag="mut")
                lvt_sb = tailp.tile([P, S], F32, tag="lvt")
                clzmu_sb = tailp.tile([P, S], F32, tag="clzmu")
                var_sb = tailp.tile([P, S], F32, tag="var")
                inv_sb = tailp.tile([P, S], F32, tag="inv")
                prod_sb = tailp.tile([P, S], F32, tag="prod")
                explv_sb = tailp.tile([P, S], F32, tag="explv")
                zt_sb = tailp.tile([P, S], F32, tag="zt")
                ztbf_sb = tailp.tile([P, S], BF16, tag="ztbf")
                ubloc_sb = tailp.tile([P, S], BF16, tag="ubloc")
                uzloc_sb = tailp.tile([P, S], BF16, tag="uzloc")
                ubfull_sb = tailp.tile([P, N], BF16, tag="ubfull")
                uzfull_sb = tailp.tile([P, N], BF16, tag="uzfull")
                stats_sb = tailp.tile([P, 2], F32, tag="stats")
                statsg_sb = tailp.tile([P, 2], F32, tag="statsg")

                def m4_block(fb, ptag, pname):
                    ps = psacc.tile([P, S], F32, tag=ptag, name=pname)
                    for jc in range(2):
                        for hf in range(2):
                            nc.tensor.matmul(
                                ps[:, hf * 512:(hf + 1) * 512],
                                lhsT=wcat_sb[:, jc * 512 + fb * P: jc * 512 + (fb + 1) * P],
                                rhs=sT_sb[:, jc * S + hf * 512: jc * S + (hf + 1) * 512],
                                start=(jc == 0),
                                stop=(jc == 1),
                            )
                    return ps

                # clz stats path first: it is the longer chain feeding the
                # packed all-gather input
                ps = m4_block(3, "accT0", "m4cv")
                nc.scalar.activation(var_sb[:], ps[:], AF.Exp)
                ps = m4_block(2, "accT1", "m4cm")
                nc.scalar.copy(clzmu_sb[:], ps[:])
                nc.vector.reciprocal_approx_fast(inv_sb[:], var_sb[:])
                nc.vector.tensor_mul(prod_sb[:], clzmu_sb[:], inv_sb[:])
                nc.vector.reduce_sum(stats_sb[:, 0:1], inv_sb[:], axis=AX)
                nc.vector.reduce_sum(stats_sb[:, 1:2], prod_sb[:], axis=AX)
                sthi_bf = tailp.tile([P, 2], BF16, tag="sthi")
                sthi_f = tailp.tile([P, 2], F32, tag="sthif")
                stlo_bf = tailp.tile([P, 2], BF16, tag="stlo")
                nc.vector.tensor_copy(sthi_bf[:], stats_sb[:])
                nc.vector.tensor_copy(sthi_f[:], sthi_bf[:])
                nc.vector.tensor_sub(sthi_f[:], stats_sb[:], sthi_f[:])
                nc.vector.tensor_copy(stlo_bf[:], sthi_f[:])
                nc.sync.dma_start(uz_bnc[:, S:S + 2], sthi_bf[:])
                nc.sync.dma_start(uz_bnc[:, S + 2:S + 4], stlo_bf[:])

                ps = m4_block(0, "accT0", "m4mu")
                nc.scalar.copy(mut_sb[:], ps[:])
                ps = m4_block(1, "accT1", "m4lv")
                nc.scalar.copy(lvt_sb[:], ps[:])
                nc.sync.dma_start(mut_o[:, :], mut_sb[:])
                nc.sync.dma_start(lvt_o[:, :], lvt_sb[:])

                # z = eps_z * exp(logvar) + mu (transposed layout)
                nc.scalar.activation(explv_sb[:], lvt_sb[:], AF.Exp)
                nc.vector.tensor_mul(zt_sb[:], epszt_sb[:], explv_sb[:])
                nc.vector.tensor_add(zt_sb[:], zt_sb[:], mut_sb[:])
                nc.sync.dma_start(zt_o[:, :], zt_sb[:])
                nc.scalar.copy(ztbf_sb[:], zt_sb[:])

                # ub = WlA^T @ zT (const row applied post-gather)
                ups = psacc.tile([P, S], F32, tag="accT0", name="ups")
                for hf in range(2):
                    nc.tensor.matmul(
                        ups[:, hf * 512:(hf + 1) * 512],
                        lhsT=wla_sb[:],
                        rhs=ztbf_sb[:, hf * 512:(hf + 1) * 512],
                        start=True,
                        stop=True,
                    )
                nc.vector.tensor_copy(ubloc_sb[:], ups[:])
                nc.sync.dma_start(uz_bnc[:, :S], ubloc_sb[:])

                nc.gpsimd.collective_compute(
                    "AllGather", mybir.AluOpType.bypass, replica_groups=RG,
                    ins=[uz_bnc[:].opt()], outs=[uz_g[:].opt()],
                )

                # gather-in: ub slices (scalar/sync queues) + packed stats
                for r in range(NCORE):
                    eng = nc.scalar if r % 2 == 0 else nc.sync
                    eng.dma_start(
                        ubfull_sb[:, r * S:(r + 1) * S],
                        uz_g[r * H2:(r + 1) * H2, :S],
                    )
                stpack_sb = tailp.tile([P, NCORE * 4], BF16, tag="stpack")
                nc.scalar.dma_start(
                    stpack_sb[:].rearrange("p (r c) -> p r c", r=NCORE),
                    uz_g[:, S:S + 4].rearrange("(r p) c -> p r c", p=P),
                )
                # statsg = sum over ranks of (hi + lo): one strided reduce over
                # the rank axis, then fold hi+lo
                st4_sb = tailp.tile([P, 4], F32, tag="st4")
                nc.vector.tensor_reduce(
                    st4_sb[:],
                    stpack_sb[:].rearrange("p (r c) -> p c r", c=4),
                    axis=AX, op=mybir.AluOpType.add,
                )
                nc.vector.tensor_add(statsg_sb[:], st4_sb[:, 0:2], st4_sb[:, 2:4])

                # group math (tiny, [128,1] columns)
                gv_sb = tailp.tile([P, 1], F32, tag="gv")
                gmu_sb = tailp.tile([P, 1], F32, tag="gmu")
                glv_sb = tailp.tile([P, 1], F32, tag="glv")
                sd_sb = tailp.tile([P, 1], F32, tag="sd")
                clat_sb = tailp.tile([P, 1], F32, tag="clat")
                crow_sb = tailp.tile([P, 1], F32, tag="crow")
                grp_sb = tailp.tile([P, 2], F32, tag="grp")

                nc.vector.reciprocal(gv_sb[:], statsg_sb[:, 0:1])
                nc.vector.tensor_mul(gmu_sb[:], gv_sb[:], statsg_sb[:, 1:2])
                nc.scalar.activation(glv_sb[:], gv_sb[:], AF.Ln)
                nc.vector.tensor_copy(grp_sb[:, 0:1], gmu_sb[:])
                nc.vector.tensor_copy(grp_sb[:, 1:2], glv_sb[:])
                nc.sync.dma_start(grp_o[:, :], grp_sb[:])
                # class_lat = gmu + exp(0.5*glv) * eps_group
                nc.scalar.activation(sd_sb[:], glv_sb[:], AF.Exp, scale=0.5)
                nc.vector.tensor_mul(clat_sb[:], sd_sb[:], epsg_sb[:])
                nc.vector.tensor_add(clat_sb[:], clat_sb[:], gmu_sb[:])
                # const_row = WlB^T @ class_lat + bl   (as a [128,1] column)
                cps = pssm.tile([P, 1], F32, tag="sm", name="cps")
                nc.tensor.matmul(
                    cps[:], lhsT=wlb_sb[:], rhs=clat_sb[:], start=True, stop=True
                )
                nc.vector.tensor_add(crow_sb[:], cps[:], blc_sb[:])

                # uz = ub + const_row, applied per gathered rank block so the
                # zz matmuls below can start as each rank's data lands
                nc.vector.tensor_scalar_add(uzloc_sb[:], ubloc_sb[:], crow_sb[:])
                for r in range(NCORE):
                    blk = slice(r * S, (r + 1) * S)
                    if r % 2 == 0:
                        nc.vector.tensor_scalar_add(
                            uzfull_sb[:, blk], ubfull_sb[:, blk], crow_sb[:]
                        )
                    else:
                        nc.scalar.activation(
                            uzfull_sb[:, blk], ubfull_sb[:, blk], AF.Identity,
                            bias=crow_sb[:],
                        )

                # ---------- adj_recon row shard = uz_shard @ uz_full^T (bf16) ---
                # rank-major: columns for source rank r only need that rank's
                # gathered block, so compute overlaps the gather-in DMAs
                with nc.named_scope("zz"):
                    for r in range(NCORE):
                        for ib in range(SB):
                            rc2 = rcpool.tile([P, 1024], F32, tag="rc",
                                              name=f"rc{r}_{ib}")
                            for q in range(2):
                                nb = 2 * r + q
                                ps = pssm.tile([P, 512], F32, tag="sm",
                                               name=f"zz{r}_{ib}_{q}")
                                nc.tensor.matmul(
                                    ps[:],
                                    lhsT=uzloc_sb[:, ib * P:(ib + 1) * P],
                                    rhs=uzfull_sb[:, nb * 512:(nb + 1) * 512],
                                    start=True,
                                    stop=True,
                                )
                                if q == 0:
                                    nc.vector.tensor_copy(
                                        rc2[:, q * 512:(q + 1) * 512], ps[:]
                                    )
                                else:
                                    nc.scalar.copy(
                                        rc2[:, q * 512:(q + 1) * 512], ps[:]
                                    )
                            nc.sync.dma_start(
                                recon[ib * P:(ib + 1) * P, r * S:(r + 1) * S],
                                rc2[:],
                            )

    nc.compile()
    return nc


def _get_nc():
    global _CACHED_NC
    if _CACHED_NC is None:
        _CACHED_NC = _build()
    return _CACHED_NC


def _make_in_maps(x, adj, W1, W2, W3, W4, W5, Wl, bl, eps_z, eps_group, batch):
    assert not np.any(batch), "kernel assumes a single segment (batch all zeros)"
    adjt = np.ascontiguousarray(adj.T).astype(BF16_NP)
    xt = np.ascontiguousarray(x.T).astype(BF16_NP)
    w1 = W1.astype(BF16_NP)
    wcat = np.concatenate([W2, W3, W4, W5], axis=1).astype(BF16_NP)
    wla = np.ascontiguousarray(Wl[:H2]).astype(BF16_NP)
    wlb = np.ascontiguousarray(Wl[H2:]).astype(np.float32)
    blc_np = bl.reshape(H2, 1).astype(np.float32)
    epszt = np.ascontiguousarray(eps_z.T).astype(np.float32)
    epsgt = np.ascontiguousarray(eps_group.T).astype(np.float32)

    in_maps = []
    for c in range(NCORE):
        sl = slice(c * S, (c + 1) * S)
        in_maps.append(
            dict(
                adjt=np.ascontiguousarray(adjt[:, sl]),
                xt=xt,
                w1=w1,
                wcat=wcat,
                wla=wla,
                wlb=wlb,
                blc=blc_np,
                epszt=np.ascontiguousarray(epszt[:, sl]),
                epsgt=epsgt,
            )
        )
    return in_maps


def run_full(inputs, trace=False, **trace_kwargs):
    nc = _get_nc()
    in_maps = _make_in_maps(**inputs)
    res = bass_utils.run_bass_kernel_spmd(
        nc, in_maps, core_ids=list(range(NCORE)), trace=trace, **trace_kwargs
    )
    outs = res.results
    adj_recon = np.concatenate([outs[c]["recon"] for c in range(NCORE)], axis=0)
    z = np.concatenate([outs[c]["zt"].T for c in range(NCORE)], axis=0)
    mu = np.concatenate([outs[c]["mut"].T for c in range(NCORE)], axis=0)
    logvar = np.concatenate([outs[c]["lvt"].T for c in range(NCORE)], axis=0)
    grp = outs[0]["grp"]
    grouped_mu = np.ascontiguousarray(
        np.broadcast_to(grp[:, 0][None, :], (N, H2)).astype(np.float32)
    )
    grouped_logvar = np.ascontiguousarray(
        np.broadcast_to(grp[:, 1][None, :], (N, H2)).astype(np.float32)
    )
    return (adj_recon, z, mu, logvar, grouped_mu, grouped_logvar), res


def kernel(**inputs):
    return run_full(inputs, trace=False)[0]


# revision 25
# speedup vs baseline: 1.0818x; 1.0818x over previous
"""GCN-VAE forward as a distributed 8-core Trainium2 Bass kernel.

Sharding: nodes (rows of adj / all [N,*] activations) are split across the
8 cores, 1024 rows each.  adj and x are fed pre-transposed (adjT / xT column
shards) so every matmul contracts over the partition axis without on-the-fly
DRAM transposes.  Heavy matmuls run in bf16 with fp32 PSUM accumulation.

Structure per core:
  warmup AllGather (absorbs inter-core start skew) || adjT resident load
  pass0: xw = x@W1 shard -> AllGather
  pass1: h1T = (xw^T contracted with adjT) -> relu -> PE-transpose -> AllGather
  pass2: sT = (adj@h1)^T shard;  m4T = Wcat^T . sT  (mu/logvar/clz_mu/clz_logvar)
  z -> ub = WlA^T zT -> AllGather(ub)  [issued before the stats AllReduce so the
  reduce hides under the gather];  uz = ub + const_row applied post-gather
  zz: adj_recon row shard = uz_shard @ uz_full^T (bf16)

Device outputs per core (host reassembles / transposes):
  recon [1024, 8192] f32; zt/mut/lvt [128, 1024] f32; grp [128, 2] f32
"""

import sys

for _p in ("/opt/trn_rl_repo",):
    if _p not in sys.path:
        sys.path.insert(0, _p)

import numpy as np
import ml_dtypes

import concourse.bass as bass
import concourse.bacc as bacc
import concourse.tile as tile
import concourse.mybir as mybir
from concourse import bass_utils, masks

F32 = mybir.dt.float32
BF16 = mybir.dt.bfloat16
BF16_NP = ml_dtypes.bfloat16

N = 8192          # nodes
NCORE = 8
S = N // NCORE    # 1024 rows per core
FIN = 512
H1 = 256
H2 = 128
P = 128           # partitions
KCH = N // P      # 64 contraction chunks over nodes
SB = S // P       # 8 row blocks per core
RES = 44          # adjT chunks kept resident in SBUF (rest streamed per pass)

_CACHED_NC = None


def _build():
    nc = bacc.Bacc(
        "TRN2",
        target_bir_lowering=False,
        debug=False,
        enable_asserts=True,
        num_devices=NCORE,
    )

    # ---- per-core external inputs ----
    adjt = nc.dram_tensor("adjt", [N, S], BF16, kind="ExternalInput")
    xt = nc.dram_tensor("xt", [FIN, N], BF16, kind="ExternalInput")
    w1 = nc.dram_tensor("w1", [FIN, H1], BF16, kind="ExternalInput")
    wcat = nc.dram_tensor("wcat", [H1, 4 * H2], BF16, kind="ExternalInput")
    wla = nc.dram_tensor("wla", [H2, H2], BF16, kind="ExternalInput")
    wlb = nc.dram_tensor("wlb", [H2, H2], F32, kind="ExternalInput")
    blc = nc.dram_tensor("blc", [H2, 1], F32, kind="ExternalInput")
    epszt = nc.dram_tensor("epszt", [H2, S], F32, kind="ExternalInput")
    epsgt = nc.dram_tensor("epsgt", [H2, 1], F32, kind="ExternalInput")

    # ---- per-core external outputs ----
    recon = nc.dram_tensor("recon", [S, N], F32, kind="ExternalOutput")
    zt_o = nc.dram_tensor("zt", [H2, S], F32, kind="ExternalOutput")
    mut_o = nc.dram_tensor("mut", [H2, S], F32, kind="ExternalOutput")
    lvt_o = nc.dram_tensor("lvt", [H2, S], F32, kind="ExternalOutput")
    grp_o = nc.dram_tensor("grp", [H2, 2], F32, kind="ExternalOutput")

    RG = [list(range(NCORE))]
    AX = mybir.AxisListType.X
    AF = mybir.ActivationFunctionType

    with tile.TileContext(nc) as tc:
        with (
            tc.tile_pool(name="const", bufs=1) as constp,
            tc.tile_pool(name="persist", bufs=1) as persist,
            tc.tile_pool(name="drain", bufs=3) as drainp,
            tc.tile_pool(name="psacc", bufs=1, space="PSUM") as psacc,
            tc.tile_pool(name="pssm", bufs=4, space="PSUM") as pssm,
            tc.tile_pool(name="dram", bufs=1, space="DRAM") as dramp,
        ):
            # collective buffers
            h1_bnc = dramp.tile([S, H1], BF16, tag="h1_bnc")
            h1_g = dramp.tile([N, H1], BF16, tag="h1_g", addr_space="Shared")
            uz_bnc = dramp.tile([H2, S + 4], BF16, tag="uz_bnc")
            uz_g = dramp.tile([NCORE * H2, S + 4], BF16, tag="uz_g",
                              addr_space="Shared")

            # big feature slot reused xw_full -> h1_full
            def bigfeat(dtype, cols, name):
                return persist.tile([P, cols], dtype, tag="bigfeat", name=name)

            h1t_sb = [
                persist.tile([P, S], BF16, tag=f"h1t{jb}", name=f"h1t_sb{jb}")
                for jb in range(2)
            ]
            sT_sb = persist.tile([P, 2 * S], BF16, tag="sT")

            with (
                tc.tile_pool(name="adjres", bufs=1) as adjp,
                tc.tile_pool(name="adjstream", bufs=12) as adjsp,
            ):
                # ---------- adjT resident load first, on the gpsimd DGE ----------
                adj_res = []
                with nc.named_scope("adj_load"):
                    for c in range(RES):
                        t = adjp.tile([P, S], BF16, tag=f"adj{c}", name=f"adj{c}")
                        nc.gpsimd.dma_start(t[:], adjt[c * P:(c + 1) * P, :])
                        adj_res.append(t)

                def adj_chunk(c, phase):
                    if c < RES:
                        return adj_res[c]
                    t = adjsp.tile([P, S], BF16, tag="adjstream",
                                   name=f"adjs_{phase}_{c}")
                    nc.gpsimd.dma_start(t[:], adjt[c * P:(c + 1) * P, :])
                    return t

                # ---------- constants (small, on the sync DGE) ----------
                ident_bf = constp.tile([P, P], BF16, tag="identb")
                masks.make_identity(nc, ident_bf[:])
                w1_sb = constp.tile([P, 4 * H1], BF16, tag="w1")
                for c in range(4):
                    nc.sync.dma_start(
                        w1_sb[:, c * H1:(c + 1) * H1], w1[c * P:(c + 1) * P, :]
                    )
                wcat_sb = constp.tile([P, 2 * 512], BF16, tag="wcat")
                for c in range(2):
                    nc.scalar.dma_start(
                        wcat_sb[:, c * 512:(c + 1) * 512], wcat[c * P:(c + 1) * P, :]
                    )
                wla_sb = constp.tile([P, H2], BF16, tag="wla")
                nc.scalar.dma_start(wla_sb[:], wla[:, :])
                wlb_sb = constp.tile([P, H2], F32, tag="wlb")
                nc.scalar.dma_start(wlb_sb[:], wlb[:, :])
                blc_sb = constp.tile([P, 1], F32, tag="blc")
                nc.scalar.dma_start(blc_sb[:], blc[:, :])
                epsg_sb = constp.tile([P, 1], F32, tag="epsg")
                nc.scalar.dma_start(epsg_sb[:], epsgt[:, :])
                epszt_sb = constp.tile([P, S], F32, tag="epszt")
                nc.scalar.dma_start(epszt_sb[:], epszt[:, :])

                # ---------- fused pass0+pass1 ----------
                # pass0 chunk c produces xw rows [c*128,(c+1)*128); pass1 chunk
                # c consumes them immediately (software-pipelined by one chunk
                # so the PE never waits on the ACT drain).
                xw_full = bigfeat(BF16, KCH * H1, "xw_full")
                h1t_ps = [
                    psacc.tile([P, S], F32, tag=f"accT{jb}", name=f"h1t_ps{jb}")
                    for jb in range(2)
                ]
                with tc.tile_pool(name="xtpool", bufs=3) as xtp:
                    NG = 8  # column groups of 1024 nodes
                    GW = N // NG
                    xt_tiles = {}

                    def xt_group(g):
                        if g in xt_tiles:
                            return xt_tiles[g]
                        xg = xtp.tile([P, 4 * GW], BF16, tag="xtg",
                                      name=f"xt{g}")
                        for c in range(4):
                            nc.sync.dma_start(
                                xg[:, c * GW:(c + 1) * GW],
                                xt[c * P:(c + 1) * P, g * GW:(g + 1) * GW],
                            )
                        xt_tiles[g] = xg
                        return xg

                    def pass0_chunk(ib):
                        g, off = divmod(ib * P, GW)
                        # prefetch this and the next group
                        xg = xt_group(g)
                        if g + 1 < NG and off == 0:
                            xt_group(g + 1)
                        ps = pssm.tile([P, H1], F32, tag="sm", name=f"ps0_{ib}")
                        for kc in range(4):
                            nc.tensor.matmul(
                                ps[:],
                                lhsT=xg[:, kc * GW + off: kc * GW + off + P],
                                rhs=w1_sb[:, kc * H1:(kc + 1) * H1],
                                start=(kc == 0),
                                stop=(kc == 3),
                            )
                        nc.scalar.copy(xw_full[:, ib * H1:(ib + 1) * H1], ps[:])

                    def pass1_chunk(c):
                        a = adj_chunk(c, 1)
                        for jb in range(2):
                            for hf in range(2):
                                nc.tensor.matmul(
                                    h1t_ps[jb][:, hf * 512:(hf + 1) * 512],
                                    lhsT=xw_full[:, c * H1 + jb * P: c * H1 + (jb + 1) * P],
                                    rhs=a[:, hf * 512:(hf + 1) * 512],
                                    start=(c == 0),
                                    stop=(c == KCH - 1),
                                )

                    with nc.named_scope("p01"):
                        for c in range(KCH + 1):
                            if c < KCH:
                                pass0_chunk(c)
                            if c >= 1:
                                pass1_chunk(c - 1)

                for jb in range(2):
                    nc.scalar.activation(h1t_sb[jb][:], h1t_ps[jb][:], AF.Relu)

                # transpose h1T -> h1 natural [1024, 256], all-gather
                for b in range(SB):
                    h1n = drainp.tile([P, H1], BF16, tag="h1n", name=f"h1n{b}")
                    for jb in range(2):
                        tp = pssm.tile([P, P], BF16, tag="sm", name=f"tp{b}_{jb}")
                        nc.tensor.transpose(
                            tp[:], h1t_sb[jb][:, b * P:(b + 1) * P], ident_bf[:]
                        )
                        if jb == 0:
                            nc.vector.tensor_copy(h1n[:, jb * P:(jb + 1) * P], tp[:])
                        else:
                            nc.scalar.copy(h1n[:, jb * P:(jb + 1) * P], tp[:])
                    nc.sync.dma_start(h1_bnc[b * P:(b + 1) * P, :], h1n[:])

                nc.gpsimd.collective_compute(
                    "AllGather", mybir.AluOpType.bypass, replica_groups=RG,
                    ins=[h1_bnc[:].opt()], outs=[h1_g[:].opt()],
                )
                h1_full = bigfeat(BF16, KCH * H1, "h1_full")
                for b in range(8):
                    srcap = h1_g[b * 8 * P:(b + 1) * 8 * P, :].rearrange(
                        "(c p) j -> p c j", p=P
                    )
                    dstap = h1_full[:, b * 8 * H1:(b + 1) * 8 * H1].rearrange(
                        "p (c j) -> p c j", c=8
                    )
                    eng = (nc.scalar, nc.sync, nc.gpsimd)[b % 3]
                    eng.dma_start(dstap, srcap)

                # ---------- pass 2: sT[j,i] = sum_k h1[k,j] adjT[k,i] ----------
                st_ps = [
                    psacc.tile([P, S], F32, tag=f"accT{jb}", name=f"st_ps{jb}")
                    for jb in range(2)
                ]
                with nc.named_scope("pass2"):
                    for c in range(KCH):
                        a = adj_chunk(c, 2)
                        for jb in range(2):
                            for hf in range(2):
                                nc.tensor.matmul(
                                    st_ps[jb][:, hf * 512:(hf + 1) * 512],
                                    lhsT=h1_full[:, c * H1 + jb * P: c * H1 + (jb + 1) * P],
                                    rhs=a[:, hf * 512:(hf + 1) * 512],
                                    start=(c == 0),
                                    stop=(c == KCH - 1),
                                )
                for jb in range(2):
                    nc.scalar.copy(sT_sb[:, jb * S:(jb + 1) * S], st_ps[jb][:])

            # ---------- tail ----------
            with (
                tc.tile_pool(name="tail", bufs=1) as tailp,
                tc.tile_pool(name="rcpool", bufs=3) as rcpool,
            ):
                mut_sb = tailp.tile([P, S], F32, tag="mut")
                lvt_sb = tailp.tile([P, S], F32, tag="lvt")
                clzmu_sb = tailp.tile([P, S], F32, tag="clzmu")
                var_sb = tailp.tile([P, S], F32, tag="var")
                inv_sb = tailp.tile([P, S], F32, tag="inv")
                prod_sb = tailp.tile([P, S], F32, tag="prod")
                explv_sb = tailp.tile([P, S], F32, tag="explv")
                zt_sb = tailp.tile([P, S], F32, tag="zt")
                ztbf_sb = tailp.tile([P, S], BF16, tag="ztbf")
                ubloc_sb = tailp.tile([P, S], BF16, tag="ubloc")
                uzloc_sb = tailp.tile([P, S], BF16, tag="uzloc")
                ubfull_sb = tailp.tile([P, N], BF16, tag="ubfull")
                uzfull_sb = tailp.tile([P, N], BF16, tag="uzfull")
                stats_sb = tailp.tile([P, 2], F32, tag="stats")
                statsg_sb = tailp.tile([P, 2], F32, tag="statsg")

                def m4_block(fb, ptag, pname):
                    ps = psacc.tile([P, S], F32, tag=ptag, name=pname)
                    for jc in range(2):
                        for hf in range(2):
                            nc.tensor.matmul(
                                ps[:, hf * 512:(hf + 1) * 512],
                                lhsT=wcat_sb[:, jc * 512 + fb * P: jc * 512 + (fb + 1) * P],
                                rhs=sT_sb[:, jc * S + hf * 512: jc * S + (hf + 1) * 512],
                                start=(jc == 0),
                                stop=(jc == 1),
                            )
                    return ps

                # clz stats path first: it is the longer chain feeding the
                # packed all-gather input
                ps = m4_block(3, "accT0", "m4cv")
                nc.scalar.activation(var_sb[:], ps[:], AF.Exp)
                ps = m4_block(2, "accT1", "m4cm")
                nc.scalar.copy(clzmu_sb[:], ps[:])
                nc.vector.reciprocal_approx_fast(inv_sb[:], var_sb[:])
                nc.vector.tensor_mul(prod_sb[:], clzmu_sb[:], inv_sb[:])
                nc.vector.reduce_sum(stats_sb[:, 0:1], inv_sb[:], axis=AX)
                nc.vector.reduce_sum(stats_sb[:, 1:2], prod_sb[:], axis=AX)
                sthi_bf = tailp.tile([P, 2], BF16, tag="sthi")
                sthi_f = tailp.tile([P, 2], F32, tag="sthif")
                stlo_bf = tailp.tile([P, 2], BF16, tag="stlo")
                nc.vector.tensor_copy(sthi_bf[:], stats_sb[:])
                nc.vector.tensor_copy(sthi_f[:], sthi_bf[:])
                nc.vector.tensor_sub(sthi_f[:], stats_sb[:], sthi_f[:])
                nc.vector.tensor_copy(stlo_bf[:], sthi_f[:])
                nc.sync.dma_start(uz_bnc[:, S:S + 2], sthi_bf[:])
                nc.sync.dma_start(uz_bnc[:, S + 2:S + 4], stlo_bf[:])

                ps = m4_block(0, "accT0", "m4mu")
                nc.scalar.copy(mut_sb[:], ps[:])
                ps = m4_block(1, "accT1", "m4lv")
                nc.scalar.copy(lvt_sb[:], ps[:])
                nc.sync.dma_start(mut_o[:, :], mut_sb[:])
                nc.sync.dma_start(lvt_o[:, :], lvt_sb[:])

                # z = eps_z * exp(logvar) + mu (transposed layout)
                nc.scalar.activation(explv_sb[:], lvt_sb[:], AF.Exp)
                nc.vector.tensor_mul(zt_sb[:], epszt_sb[:], explv_sb[:])
                nc.vector.tensor_add(zt_sb[:], zt_sb[:], mut_sb[:])
                nc.sync.dma_start(zt_o[:, :], zt_sb[:])
                nc.scalar.copy(ztbf_sb[:], zt_sb[:])

                # ub = WlA^T @ zT (const row applied post-gather)
                ups = psacc.tile([P, S], F32, tag="accT0", name="ups")
                for hf in range(2):
                    nc.tensor.matmul(
                        ups[:, hf * 512:(hf + 1) * 512],
                        lhsT=wla_sb[:],
                        rhs=ztbf_sb[:, hf * 512:(hf + 1) * 512],
                        start=True,
                        stop=True,
                    )
                nc.vector.tensor_copy(ubloc_sb[:], ups[:])
                nc.sync.dma_start(uz_bnc[:, :S], ubloc_sb[:])

                nc.gpsimd.collective_compute(
                    "AllGather", mybir.AluOpType.bypass, replica_groups=RG,
                    ins=[uz_bnc[:].opt()], outs=[uz_g[:].opt()],
                )

                # gather-in: ub slices (scalar/sync queues) + packed stats
                for r in range(NCORE):
                    eng = nc.scalar if r % 2 == 0 else nc.sync
                    eng.dma_start(
                        ubfull_sb[:, r * S:(r + 1) * S],
                        uz_g[r * H2:(r + 1) * H2, :S],
                    )
                stpack_sb = tailp.tile([P, NCORE * 4], BF16, tag="stpack")
                nc.scalar.dma_start(
                    stpack_sb[:].rearrange("p (r c) -> p r c", r=NCORE),
                    uz_g[:, S:S + 4].rearrange("(r p) c -> p r c", p=P),
                )
                # statsg = sum over ranks of (hi + lo): one strided reduce over
                # the rank axis, then fold hi+lo
                st4_sb = tailp.tile([P, 4], F32, tag="st4")
                nc.vector.tensor_reduce(
                    st4_sb[:],
                    stpack_sb[:].rearrange("p (r c) -> p c r", c=4),
                    axis=AX, op=mybir.AluOpType.add,
                )
                nc.vector.tensor_add(statsg_sb[:], st4_sb[:, 0:2], st4_sb[:, 2:4])

                # group math (tiny, [128,1] columns)
                gv_sb = tailp.tile([P, 1], F32, tag="gv")
                gmu_sb = tailp.tile([P, 1], F32, tag="gmu")
                glv_sb = tailp.tile([P, 1], F32, tag="glv")
                sd_sb = tailp.tile([P, 1], F32, tag="sd")
                clat_sb = tailp.tile([P, 1], F32, tag="clat")
                crow_sb = tailp.tile([P, 1], F32, tag="crow")
                grp_sb = tailp.tile([P, 2], F32, tag="grp")

                nc.vector.reciprocal(gv_sb[:], statsg_sb[:, 0:1])
                nc.vector.tensor_mul(gmu_sb[:], gv_sb[:], statsg_sb[:, 1:2])
                nc.scalar.activation(glv_sb[:], gv_sb[:], AF.Ln)
                nc.vector.tensor_copy(grp_sb[:, 0:1], gmu_sb[:])
                nc.vector.tensor_copy(grp_sb[:, 1:2], glv_sb[:])
                nc.sync.dma_start(grp_o[:, :], grp_sb[:])
                # class_lat = gmu + exp(0.5*glv) * eps_group; exp(0.5*ln(gv))
                # is just sqrt(gv), skipping the Ln dependency
                nc.scalar.activation(sd_sb[:], gv_sb[:], AF.Sqrt)
                nc.vector.tensor_mul(clat_sb[:], sd_sb[:], epsg_sb[:])
                nc.vector.tensor_add(clat_sb[:], clat_sb[:], gmu_sb[:])
                # const_row = WlB^T @ class_lat + bl   (as a [128,1] column)
                cps = pssm.tile([P, 1], F32, tag="sm", name="cps")
                nc.tensor.matmul(
                    cps[:], lhsT=wlb_sb[:], rhs=clat_sb[:], start=True, stop=True
                )
                nc.vector.tensor_add(crow_sb[:], cps[:], blc_sb[:])

                # uz = ub + const_row, applied per gathered rank block so the
                # zz matmuls below can start as each rank's data lands
                nc.vector.tensor_scalar_add(uzloc_sb[:], ubloc_sb[:], crow_sb[:])
                for r in range(NCORE):
                    blk = slice(r * S, (r + 1) * S)
                    if r % 2 == 0:
                        nc.vector.tensor_scalar_add(
                            uzfull_sb[:, blk], ubfull_sb[:, blk], crow_sb[:]
                        )
                    else:
                        nc.scalar.activation(
                            uzfull_sb[:, blk], ubfull_sb[:, blk], AF.Identity,
                            bias=crow_sb[:],
                        )

                # ---------- adj_recon row shard = uz_shard @ uz_full^T (bf16) ---
                # rank-major: columns for source rank r only need that rank's
                # gathered block, so compute overlaps the gather-in DMAs
                with nc.named_scope("zz"):
                    for r in range(NCORE):
                        for ib in range(SB):
                            rc2 = rcpool.tile([P, 1024], F32, tag="rc",
                                              name=f"rc{r}_{ib}")
                            for q in range(2):
                                nb = 2 * r + q
                                ps = pssm.tile([P, 512], F32, tag="sm",
                                               name=f"zz{r}_{ib}_{q}")
                                nc.tensor.matmul(
                                    ps[:],
                                    lhsT=uzloc_sb[:, ib * P:(ib + 1) * P],
                                    rhs=uzfull_sb[:, nb * 512:(nb + 1) * 512],
                                    start=True,
                                    stop=True,
                                )
                                if q == 0:
                                    nc.vector.tensor_copy(
                                        rc2[:, q * 512:(q + 1) * 512], ps[:]
                                    )
                                else:
                                    nc.scalar.copy(
                                        rc2[:, q * 512:(q + 1) * 512], ps[:]
                                    )
                            nc.sync.dma_start(
                                recon[ib * P:(ib + 1) * P, r * S:(r + 1) * S],
                                rc2[:],
                            )

    nc.compile()
    return nc


def _get_nc():
    global _CACHED_NC
    if _CACHED_NC is None:
        _CACHED_NC = _build()
    return _CACHED_NC


def _make_in_maps(x, adj, W1, W2, W3, W4, W5, Wl, bl, eps_z, eps_group, batch):
    assert not np.any(batch), "kernel assumes a single segment (batch all zeros)"
    adjt = np.ascontiguousarray(adj.T).astype(BF16_NP)
    xt = np.ascontiguousarray(x.T).astype(BF16_NP)
    w1 = W1.astype(BF16_NP)
    wcat = np.concatenate([W2, W3, W4, W5], axis=1).astype(BF16_NP)
    wla = np.ascontiguousarray(Wl[:H2]).astype(BF16_NP)
    wlb = np.ascontiguousarray(Wl[H2:]).astype(np.float32)
    blc_np = bl.reshape(H2, 1).astype(np.float32)
    epszt = np.ascontiguousarray(eps_z.T).astype(np.float32)
    epsgt = np.ascontiguousarray(eps_group.T).astype(np.float32)

    in_maps = []
    for c in range(NCORE):
        sl = slice(c * S, (c + 1) * S)
        in_maps.append(
            dict(
                adjt=np.ascontiguousarray(adjt[:, sl]),
                xt=xt,
                w1=w1,
                wcat=wcat,
                wla=wla,
                wlb=wlb,
                blc=blc_np,
                epszt=np.ascontiguousarray(epszt[:, sl]),
                epsgt=epsgt,
            )
        )
    return in_maps


def run_full(inputs, trace=False, **trace_kwargs):
    nc = _get_nc()
    in_maps = _make_in_maps(**inputs)
    res = bass_utils.run_bass_kernel_spmd(
        nc, in_maps, core_ids=list(range(NCORE)), trace=trace, **trace_kwargs
    )
    outs = res.results
    adj_recon = np.concatenate([outs[c]["recon"] for c in range(NCORE)], axis=0)
    z = np.concatenate([outs[c]["zt"].T for c in range(NCORE)], axis=0)
    mu = np.concatenate([outs[c]["mut"].T for c in range(NCORE)], axis=0)
    logvar = np.concatenate([outs[c]["lvt"].T for c in range(NCORE)], axis=0)
    grp = outs[0]["grp"]
    grouped_mu = np.ascontiguousarray(
        np.broadcast_to(grp[:, 0][None, :], (N, H2)).astype(np.float32)
    )
    grouped_logvar = np.ascontiguousarray(
        np.broadcast_to(grp[:, 1][None, :], (N, H2)).astype(np.float32)
    )
    return (adj_recon, z, mu, logvar, grouped_mu, grouped_logvar), res


def kernel(**inputs):
    return run_full(inputs, trace=False)[0]
